# revision 3
# baseline (speedup 1.0000x reference)
"""Distributed Trainium2 kernel for 8-head MHA with axial (2D) RoPE.

Problem: x:(2,4096,512) f32, Wq/Wk/Wv/Wo:(512,512), T=128, V=32.
  q/k/v = x @ W.T split into 8 heads of 64; q,k get axial rope
  (first 32 chans rotated by angle t_idx=s//V, next 32 by v_idx=s%V,
  interleaved-pair convention); dense softmax attention; out proj.

Sharding (8 cores): core c -> batch b=c//4, head pair (2*(c%4), 2*(c%4)+1).
Each core computes the full attention for its two heads and a partial
output projection over its 128 channels; the host sums the 4 partials
per batch.

Per-core kernel (all matmuls bf16, accumulation f32):
  - host supplies x^T, W^T slices, rope cos/sin tables pre-arranged
  - projections produce q^T,k^T,v^T (channels on partitions)
  - rope applied in natural layout between two bf16 DMA-transposes
  - attention: scores^T = k^T.T @ q^T per 128-key tile (both heads packed
    into one PE pass via row tile_position), exp on ScalarE straight from
    PSUM (scale=1/8 fused), PV matmul with a ones column appended to V so
    PSUM row 64 accumulates the softmax denominator
  - denominators bounce through DRAM to transpose into [s-partition]
    layout; reciprocal; applied per-partition after the out projection
"""

import numpy as np
import ml_dtypes

B, S, D, H, HD = 2, 4096, 512, 8, 64
ROT_T = ROT_V = 32
ROPE_BASE = 10000.0
NCORES = 8
P = 128
CHUNK = 512  # sq chunk (psum bank)

_cache = {}


def _install_drain_patch():
    """This walrus build allows only one sync-wait on a CTRL instruction;
    Tile's tail drain carries one wait per live semaphore. Move the extra
    waits onto dedicated SP nops (same engine, program order => same
    semantics at the following barrier)."""
    import concourse.tile as tile
    import concourse.mybir as mybir
    from concourse.tile import ScopedClock

    if getattr(tile.TileContext, "_drain_patch_installed", False):
        return

    def _drain_and_barrier(self, tick_clock, wait_clock):
        nc = self.nc
        drain_inst = nc.sync.drain()
        wait_clock.add_sem_waits(
            drain_inst.ins, ScopedClock({None: tick_clock.global_clock})
        )
        si = drain_inst.ins.sync_info
        ow = list(si.on_wait or [])
        if len(ow) > 1:
            si.on_wait = [ow[0]]
            for w in ow[1:]:
                nop = nc.sync.nop(nofuse=True)
                nop.ins.sync_info = mybir.SyncInfo(on_wait=[w], on_update=[])
        nc.all_engine_barrier()
        popped = nc._tile_sem_poison_stack.pop()
        assert popped is self._sem_poison
        nc.clear_and_free_semaphores(list(self.sems.allocated().values()))
        nc.all_engine_barrier()

    tile.TileContext._drain_and_barrier = _drain_and_barrier
    tile.TileContext._drain_patch_installed = True


def _split_multiwaits(nc):
    """core_v3 walrus allows a single sync-wait command per instruction.
    Hoist extra waits onto same-engine NOPs inserted just before."""
    import concourse.mybir as mybir

    for f in nc.m.functions:
        for blk in f.blocks:
            new = []
            changed = False
            for ins in blk.instructions:
                si = getattr(ins, "sync_info", None)
                ow = list(si.on_wait) if (si is not None and si.on_wait) else []
                eng = getattr(ins, "engine", None)
                if len(ow) > 1 and eng is not None:
                    for i, w in enumerate(ow[:-1]):
                        new.append(
                            mybir.InstNoOp(
                                name=f"{ins.name}-sw{i}",
                                engine=eng,
                                sync_info=mybir.SyncInfo(
                                    on_wait=[w], on_update=[]
                                ),
                                bass_nofuse=True,
                            )
                        )
                    si.on_wait = [ow[-1]]
                    changed = True
                new.append(ins)
            if changed:
                blk.instructions = new


def _build(s_len):
    import concourse.bass as bass
    import concourse.tile as tile
    import concourse.mybir as mybir
    from concourse.bass import ts

    _install_drain_patch()

    f32 = mybir.dt.float32
    bf16 = mybir.dt.bfloat16
    NT = s_len // P        # 128-row tiles (also key tiles)
    NCH = s_len // CHUNK   # 512-wide query chunks
    TPC = CHUNK // P       # s-tiles per chunk

    nc = bass.Bass()
    xT = nc.dram_tensor("xT", [P, 4, s_len], bf16, kind="ExternalInput")
    wqT = nc.dram_tensor("wqT", [P, 4, P], bf16, kind="ExternalInput")
    wkT = nc.dram_tensor("wkT", [P, 4, P], bf16, kind="ExternalInput")
    wvT = nc.dram_tensor("wvT", [P, 4, P], bf16, kind="ExternalInput")
    woT = nc.dram_tensor("woT", [HD, 2, D], bf16, kind="ExternalInput")
    ctab = nc.dram_tensor("ctab", [P, NT, ROT_T], f32, kind="ExternalInput")
    stab = nc.dram_tensor("stab", [P, NT, ROT_T], f32, kind="ExternalInput")
    yp = nc.dram_tensor("yp", [s_len, D], f32, kind="ExternalOutput")

    with tile.TileContext(nc) as tc:
        with (
            tc.tile_pool(name="const", bufs=1) as cpool,
            tc.tile_pool(name="stage", bufs=2) as stage,
            tc.tile_pool(name="ropet", bufs=2) as ropet,
            tc.tile_pool(name="expp", bufs=3) as expp,
            tc.tile_pool(name="dch", bufs=2) as dch,
            tc.tile_pool(name="outs", bufs=3) as outs,
            tc.tile_pool(name="qkps", bufs=2, space="PSUM") as qkps,
            tc.tile_pool(name="pvps", bufs=4, space="PSUM") as pvps,
            tc.tile_pool(name="dram", bufs=1, space="DRAM") as dram,
        ):
            # ---- load constants ----
            xT_sb = cpool.tile([P, 4, s_len], bf16)
            nc.sync.dma_start(xT_sb[:], xT[:])
            wq_sb = cpool.tile([P, 4, P], bf16)
            nc.sync.dma_start(wq_sb[:], wqT[:])
            wk_sb = cpool.tile([P, 4, P], bf16)
            nc.sync.dma_start(wk_sb[:], wkT[:])
            wv_sb = cpool.tile([P, 4, P], bf16)
            nc.sync.dma_start(wv_sb[:], wvT[:])
            wo_sb = cpool.tile([HD, 2, D], bf16)
            nc.sync.dma_start(wo_sb[:], woT[:])
            ct_sb = cpool.tile([P, NT, ROT_T], f32)
            nc.sync.dma_start(ct_sb[:], ctab[:])
            st_sb = cpool.tile([P, NT, ROT_T], f32)
            nc.sync.dma_start(st_sb[:], stab[:])

            # persistent activations
            qT_pre = cpool.tile([P, s_len], bf16)   # q^T before rope
            kT_pre = cpool.tile([P, s_len], bf16)
            vT_sb = cpool.tile([P, s_len], bf16)    # v^T
            qT = cpool.tile([P, s_len], bf16)       # q^T after rope
            kT = cpool.tile([P, s_len], bf16)
            v_sb = cpool.tile([P, NT, 2, HD + 1], bf16)  # v natural + ones col
            yT0 = cpool.tile([HD, s_len], bf16)     # attention out^T head 0
            yT1 = cpool.tile([HD, s_len], bf16)
            den_dram = dram.tile([2, s_len], f32)

            nc.vector.memset(v_sb[:, :, :, HD : HD + 1], 1.0)

            # ---- projections (channels on partitions) ----
            for c in range(NCH):
                for name, w, dst in (
                    ("q", wq_sb, qT_pre),
                    ("k", wk_sb, kT_pre),
                    ("v", wv_sb, vT_sb),
                ):
                    ps = qkps.tile([P, 2, CHUNK], f32, tag="qk")
                    for dt in range(4):
                        nc.tensor.matmul(
                            ps[:, 0, :],
                            lhsT=w[:, dt, :],
                            rhs=xT_sb[:, dt, ts(c, CHUNK)],
                            start=(dt == 0),
                            stop=(dt == 3),
                        )
                    nc.vector.tensor_copy(dst[:, ts(c, CHUNK)], ps[:, 0, :])

            # ---- rope on q/k (natural layout via DMA transpose) ----
            for src, dst in ((qT_pre, qT), (kT_pre, kT)):
                nat = stage.tile([P, NT, P], bf16, tag="nat")
                for t in range(NT):
                    nc.sync.dma_start(
                        nat[:, t, :], src[:, ts(t, P)], transpose=True
                    )
                v5 = nat.rearrange("p t (h f two) -> p t h f two", h=2, two=2)
                xe = v5[:, :, :, :, 0]
                xo = v5[:, :, :, :, 1]
                cb = ct_sb[:, :, None, :].to_broadcast([P, NT, 2, ROT_T])
                sb = st_sb[:, :, None, :].to_broadcast([P, NT, 2, ROT_T])
                t1 = ropet.tile([P, NT, 2, ROT_T], bf16, tag="t1")
                t2 = ropet.tile([P, NT, 2, ROT_T], bf16, tag="t2")
                t3 = ropet.tile([P, NT, 2, ROT_T], bf16, tag="t3")
                t4 = ropet.tile([P, NT, 2, ROT_T], bf16, tag="t4")
                mul = mybir.AluOpType.mult
                nc.vector.tensor_tensor(t1[:], xe, cb, mul)
                nc.vector.tensor_tensor(t2[:], xo, sb, mul)
                nc.vector.tensor_tensor(t3[:], xe, sb, mul)
                nc.vector.tensor_tensor(t4[:], xo, cb, mul)
                nc.vector.tensor_tensor(xe, t1[:], t2[:], mybir.AluOpType.subtract)
                nc.vector.tensor_tensor(xo, t4[:], t3[:], mybir.AluOpType.add)
                for t in range(NT):
                    nc.sync.dma_start(
                        dst[:, ts(t, P)], nat[:, t, :], transpose=True
                    )

            # v: transpose v^T into natural layout [s, head, chan]
            for t in range(NT):
                vtmp = stage.tile([P, P], bf16, tag="vtmp")
                nc.sync.dma_start(vtmp[:], vT_sb[:, ts(t, P)], transpose=True)
                nc.vector.tensor_copy(
                    v_sb[:, t, :, 0:HD],
                    vtmp.rearrange("p (h c) -> p h c", h=2),
                )

            # ---- attention + output projection ----
            exp_f = mybir.ActivationFunctionType.Exp
            scale = HD ** -0.5
            for c in range(NCH):
                pv0 = pvps.tile([P, CHUNK], f32, tag="pv")
                pv1 = pvps.tile([P, CHUNK], f32, tag="pv")
                for t in range(NT):
                    qk = qkps.tile([P, 2, CHUNK], f32, tag="qk")
                    nc.tensor.matmul(
                        qk[:, 0, :],
                        lhsT=kT[0:HD, ts(t, P)],
                        rhs=qT[0:HD, ts(c, CHUNK)],
                        start=True,
                        stop=True,
                        tile_position=(0, 0),
                    )
                    nc.tensor.matmul(
                        qk[:, 1, :],
                        lhsT=kT[HD:P, ts(t, P)],
                        rhs=qT[HD:P, ts(c, CHUNK)],
                        start=True,
                        stop=True,
                        tile_position=(HD, 0),
                    )
                    ep = expp.tile([P, 2, CHUNK], bf16, tag="ep")
                    nc.scalar.activation(ep[:], qk[:], exp_f, scale=scale)
                    nc.tensor.matmul(
                        pv0[0 : HD + 1, :],
                        lhsT=v_sb[:, t, 0, :],
                        rhs=ep[:, 0, :],
                        start=(t == 0),
                        stop=(t == NT - 1),
                    )
                    nc.tensor.matmul(
                        pv1[0 : HD + 1, :],
                        lhsT=v_sb[:, t, 1, :],
                        rhs=ep[:, 1, :],
                        start=(t == 0),
                        stop=(t == NT - 1),
                    )
                # y^T and denominators out of PSUM
                nc.vector.tensor_copy(yT0[:, ts(c, CHUNK)], pv0[0:HD, :])
                nc.vector.tensor_copy(yT1[:, ts(c, CHUNK)], pv1[0:HD, :])
                dt_sb = dch.tile([HD + 1, 2, CHUNK], f32, tag="den")
                nc.vector.tensor_copy(dt_sb[HD : HD + 1, 0, :], pv0[HD : HD + 1, :])
                nc.vector.tensor_copy(dt_sb[HD : HD + 1, 1, :], pv1[HD : HD + 1, :])
                nc.sync.dma_start(
                    den_dram[0, ts(c, CHUNK)], dt_sb[HD : HD + 1, 0, :]
                )
                nc.sync.dma_start(
                    den_dram[1, ts(c, CHUNK)], dt_sb[HD : HD + 1, 1, :]
                )
                # transpose denominators into [s-partition] layout + recip
                rt = dch.tile([P, 2, TPC], f32, tag="rt")
                nc.sync.dma_start(
                    rt[:],
                    den_dram[:, ts(c, CHUNK)].rearrange("h (t p) -> p h t", p=P),
                )
                rr = dch.tile([P, 2, TPC], f32, tag="rr")
                nc.vector.reciprocal(rr[:], rt[:])
                # output projection per 128-row s-tile, scaled by 1/den
                for st in range(TPC):
                    sg = c * TPC + st
                    op0 = pvps.tile([P, CHUNK], f32, tag="pv")
                    nc.tensor.matmul(
                        op0[:],
                        lhsT=yT0[:, ts(sg, P)],
                        rhs=wo_sb[:, 0, :],
                        start=True,
                        stop=True,
                    )
                    acc = outs.tile([P, D], f32, tag="acc")
                    nc.vector.tensor_scalar_mul(acc[:], op0[:], rr[:, 0, st : st + 1])
                    op1 = pvps.tile([P, CHUNK], f32, tag="pv")
                    nc.tensor.matmul(
                        op1[:],
                        lhsT=yT1[:, ts(sg, P)],
                        rhs=wo_sb[:, 1, :],
                        start=True,
                        stop=True,
                    )
                    tmp = outs.tile([P, D], f32, tag="tmp")
                    nc.vector.tensor_scalar_mul(tmp[:], op1[:], rr[:, 1, st : st + 1])
                    nc.vector.tensor_tensor(
                        acc[:], acc[:], tmp[:], mybir.AluOpType.add
                    )
                    nc.sync.dma_start(yp[ts(sg, P), :], acc[:])
    _split_multiwaits(nc)
    return nc


def _host_inputs(x, Wq, Wk, Wv, Wo, V, s_len):
    """Build the 8 per-core input dicts."""
    bf = ml_dtypes.bfloat16
    x = np.asarray(x, np.float32)
    Wq = np.asarray(Wq, np.float32)
    Wk = np.asarray(Wk, np.float32)
    Wv = np.asarray(Wv, np.float32)
    Wo = np.asarray(Wo, np.float32)
    NT = s_len // P

    # rope tables, arranged [p, t, f] for s = t*128+p
    s = np.arange(s_len)
    half_t, half_v = ROT_T // 2, ROT_V // 2
    inv_t = 1.0 / (ROPE_BASE ** (np.arange(half_t, dtype=np.float64) / half_t))
    inv_v = 1.0 / (ROPE_BASE ** (np.arange(half_v, dtype=np.float64) / half_v))
    ang_t = (s // V)[:, None] * inv_t[None, :]
    ang_v = (s % V)[:, None] * inv_v[None, :]
    ctab = np.concatenate([np.cos(ang_t), np.cos(ang_v)], axis=1).astype(np.float32)
    stab = np.concatenate([np.sin(ang_t), np.sin(ang_v)], axis=1).astype(np.float32)
    ctab = ctab.reshape(NT, P, ROT_T).transpose(1, 0, 2).copy()
    stab = stab.reshape(NT, P, ROT_T).transpose(1, 0, 2).copy()

    xT = {}
    for b in range(B):
        t = x[b, :s_len].T.reshape(4, P, s_len).transpose(1, 0, 2)
        xT[b] = np.ascontiguousarray(t).astype(bf)

    ins = []
    for core in range(NCORES):
        b = core // 4
        hb = (core % 4) * 2 * HD
        sl = slice(hb, hb + 2 * HD)
        wqT = np.ascontiguousarray(
            Wq[sl, :].T.reshape(4, P, P).transpose(1, 0, 2)
        ).astype(bf)
        wkT = np.ascontiguousarray(
            Wk[sl, :].T.reshape(4, P, P).transpose(1, 0, 2)
        ).astype(bf)
        wvT = np.ascontiguousarray(
            Wv[sl, :].T.reshape(4, P, P).transpose(1, 0, 2)
        ).astype(bf)
        woT = np.ascontiguousarray(
            Wo[:, sl].T.reshape(2, HD, D).transpose(1, 0, 2)
        ).astype(bf)
        ins.append(
            {
                "xT": xT[b],
                "wqT": wqT,
                "wkT": wkT,
                "wvT": wvT,
                "woT": woT,
                "ctab": ctab,
                "stab": stab,
            }
        )
    return ins


def kernel(x, Wq, Wk, Wv, Wo, T, V, _trace=False):
    from concourse.bass_utils import run_bass_kernel_spmd

    V = int(V)
    s_len = np.asarray(x).shape[1]
    if s_len not in _cache:
        _cache[s_len] = _build(s_len)
    nc = _cache[s_len]

    ins = _host_inputs(x, Wq, Wk, Wv, Wo, V, s_len)
    kw = {}
    if _trace:
        kw = dict(trace=True)
    res = run_bass_kernel_spmd(nc, ins, core_ids=list(range(NCORES)), **kw)

    out = np.zeros((B, s_len, D), np.float32)
    for core in range(NCORES):
        out[core // 4] += res.results[core]["yp"]
    if _trace:
        kernel.last_result = res
    return out
